# revision 3
# baseline (speedup 1.0000x reference)
"""Trainium2 Bass kernel for the ARCS segment-reduce loss.

Computation (see reference):
  pass 1: per-class weighted segment sums over pixels (both domains)
          -> [19, 256] sums + [19] denominators, AllReduce across 8 cores
  pass 2: z = feat @ cent_safe.T, masked-softmax entropy, weighted sum
          -> scalar loss pieces, finished on host.

Sharding: data-parallel over pixels, 32768 pixels/domain/core on 8 cores.

Device dataflow per core:
  - feats stream in as bf16 (cast during DMA on the SWDGE path)
  - one-hot weight matrix built per 128-pixel block with one fused
    tensor_scalar (iota == argmax) * w, used as matmul weights against the
    natural-layout feat tile -> PSUM accumulates [19, 256] + [19, 1]
  - tiny AllReduce ([19, 257] fp32) between passes
  - pass 2 re-streams feats, PE-transposes 128x128 blocks to get the
    contraction dim (D) onto partitions, z = featT.T @ centT in PSUM,
    entropy via free-dim reductions, weighted into a [128,1] accumulator.

Host finishes: centroids = sums/denoms (fp32, same as reference math) and
loss = -(sum of per-core partials) / (n_masked + N).
"""

import numpy as np

NUM_CLASS = 19
D_FEAT = 256
N_PIX = 262144
N_CORES = 8
PIX_PER_CORE = N_PIX // N_CORES  # 32768
CHUNK_BLOCKS = 16  # 128-pixel blocks per DMA chunk / entropy supertile

_BUILD_CACHE = {}


def _build(npix, n_cores):
    """Trace + compile the per-core program. npix = pixels per domain per core."""
    import ml_dtypes
    import concourse.bass as bass  # noqa: F401
    import concourse.tile as tile
    from concourse import bacc, mybir

    f32 = mybir.dt.float32
    bf16 = mybir.dt.bfloat16
    EQ = mybir.AluOpType.is_equal
    MUL = mybir.AluOpType.mult
    ADD = mybir.AluOpType.add
    SUB = mybir.AluOpType.subtract
    Exp = mybir.ActivationFunctionType.Exp
    Ln = mybir.ActivationFunctionType.Ln

    C = NUM_CLASS
    B = npix // 128           # 128-pixel blocks per domain
    assert npix % (128 * CHUNK_BLOCKS) == 0
    NCH = B // CHUNK_BLOCKS   # chunks (= entropy supertiles) per domain

    nc = bacc.Bacc("TRN2", target_bir_lowering=False, debug=False,
                   num_devices=n_cores)

    sfeat = nc.dram_tensor("sfeat", [npix, D_FEAT], f32, kind="ExternalInput")
    tfeat = nc.dram_tensor("tfeat", [npix, D_FEAT], f32, kind="ExternalInput")
    tconf = nc.dram_tensor("tconf", [npix], f32, kind="ExternalInput")
    sam = nc.dram_tensor("sam", [npix], mybir.dt.int32, kind="ExternalInput")
    tam = nc.dram_tensor("tam", [npix], mybir.dt.int32, kind="ExternalInput")
    smask = nc.dram_tensor("smask", [npix], mybir.dt.uint8, kind="ExternalInput")

    sred_out = nc.dram_tensor("sred", [C, D_FEAT + 1], f32, kind="ExternalOutput")
    accw_out = nc.dram_tensor("accw", [128, 1], f32, kind="ExternalOutput")

    ident_bf_d = nc.inline_tensor(np.eye(128).astype(ml_dtypes.bfloat16), "ident_bf")
    ident_f32_d = nc.inline_tensor(np.eye(128, dtype=np.float32), "ident_f32")
    iota_d = nc.inline_tensor(
        np.tile(np.arange(C, dtype=np.float32), (128, 1)), "iota_c")

    with tile.TileContext(nc) as tc:
        with (
            tc.tile_pool(name="const", bufs=1) as const_pool,
            tc.tile_pool(name="persist", bufs=1) as persist,
            tc.tile_pool(name="feat", bufs=3) as feat_pool,
            tc.tile_pool(name="oh", bufs=6) as oh_pool,
            tc.tile_pool(name="ftT", bufs=3) as ftT_pool,
            tc.tile_pool(name="ent", bufs=3) as ent_pool,
            tc.tile_pool(name="small", bufs=4) as small_pool,
            tc.tile_pool(name="psacc", bufs=1, space="PSUM") as psacc_pool,
            tc.tile_pool(name="pstr", bufs=2, space="PSUM") as pstr_pool,
            tc.tile_pool(name="psz", bufs=2, space="PSUM") as psz_pool,
            tc.tile_pool(name="dram", bufs=1, space="DRAM") as dram_pool,
        ):
            ident_bf = const_pool.tile([128, 128], bf16)
            nc.sync.dma_start(ident_bf[:], ident_bf_d[:])
            ident_f32 = const_pool.tile([128, 128], f32)
            nc.sync.dma_start(ident_f32[:], ident_f32_d[:])
            iota = const_pool.tile([128, C], f32)
            nc.sync.dma_start(iota[:], iota_d[:])
            ones_bf = const_pool.tile([128, 1], bf16)
            nc.vector.memset(ones_bf[:], 1.0)

            # ---- setup: per-pixel weights / argmax, transposed to column
            # layout (wT[:, b] = w for pixel block b) ----
            wT_s = persist.tile([128, B], f32)
            wT_t = persist.tile([128, B], f32)
            amT_s = persist.tile([128, B], f32)
            amT_t = persist.tile([128, B], f32)

            def load_columns(dst, src_ap, pre, name):
                # src_ap: flat [npix]; tiles of [rows<=128, 128] -> transpose
                nh = (B + 127) // 128
                v = src_ap.rearrange("(t p) -> t p", p=128)  # [B, 128]
                for h in range(nh):
                    rows = min(128, B - h * 128)
                    raw = small_pool.tile([128, 128], src_ap.dtype,
                                          name=f"raw_{name}_{h}")
                    nc.sync.dma_start(raw[:rows, :], v[h * 128:h * 128 + rows, :])
                    cvt = small_pool.tile([128, 128], f32, name=f"cvt_{name}_{h}")
                    pre(cvt[:rows, :], raw[:rows, :])
                    tp = pstr_pool.tile([128, 128], f32, name=f"tp_{name}_{h}", tag="tp")
                    nc.tensor.transpose(tp[:, :rows], cvt[:rows, :],
                                        ident_f32[:rows, :rows])
                    nc.vector.tensor_copy(dst[:, h * 128:h * 128 + rows],
                                          tp[:, :rows])

            load_columns(wT_s, smask[:], lambda o, i: nc.vector.tensor_copy(o, i),
                         "ws")
            load_columns(
                wT_t, tconf[:],
                lambda o, i: nc.vector.tensor_scalar(o, i, -1.0, 1.0, MUL, ADD),
                "wt")
            load_columns(amT_s, sam[:], lambda o, i: nc.vector.tensor_copy(o, i),
                         "ams")
            load_columns(amT_t, tam[:], lambda o, i: nc.vector.tensor_copy(o, i),
                         "amt")

            # ---- pass 1: segment sums ----
            acc_s = psacc_pool.tile([C, D_FEAT], f32)
            acc_d = psacc_pool.tile([C, 1], f32)

            first = True
            n_mm = 2 * B
            mm_i = 0
            for feat, amT, wT in ((sfeat, amT_s, wT_s), (tfeat, amT_t, wT_t)):
                fv = feat[:].rearrange("(c b p) d -> c p b d", p=128,
                                       b=CHUNK_BLOCKS)
                for ch in range(NCH):
                    ft = feat_pool.tile([128, CHUNK_BLOCKS, D_FEAT], bf16,
                                        name="ft1", tag="ft1")
                    nc.gpsimd.dma_start(ft[:], fv[ch])
                    for j in range(CHUNK_BLOCKS):
                        b = ch * CHUNK_BLOCKS + j
                        oh = oh_pool.tile([128, C], bf16, name="oh", tag="oh")
                        nc.vector.tensor_scalar(oh[:], iota[:], amT[:, b:b + 1],
                                                wT[:, b:b + 1], EQ, MUL)
                        mm_i += 1
                        nc.tensor.matmul(acc_s[:], oh[:], ft[:, j, :],
                                         start=first, stop=(mm_i == n_mm))
                        nc.tensor.matmul(acc_d[:], oh[:], ones_bf[:],
                                         start=first, stop=(mm_i == n_mm))
                        first = False

            sred_sb = persist.tile([C, D_FEAT + 1], f32)
            nc.vector.tensor_copy(sred_sb[:, 0:D_FEAT], acc_s[:])
            nc.vector.tensor_copy(sred_sb[:, D_FEAT:D_FEAT + 1], acc_d[:])

            # ---- AllReduce the [19, 257] partials ----
            cc_in = dram_pool.tile([C, D_FEAT + 1], f32)
            cc_addr = "Shared" if n_cores > 4 else "Local"
            cc_out = dram_pool.tile([C, D_FEAT + 1], f32, addr_space=cc_addr)
            nc.gpsimd.dma_start(cc_in[:], sred_sb[:])
            nc.gpsimd.collective_compute(
                "AllReduce",
                mybir.AluOpType.add,
                replica_groups=[list(range(n_cores))],
                ins=[cc_in.opt()],
                outs=[cc_out.opt()],
            )
            allred = persist.tile([C, D_FEAT + 1], f32)
            nc.gpsimd.dma_start(allred[:], cc_out[:])
            nc.sync.dma_start(sred_out[:], allred[:])

            # ---- centroids (bf16) for the similarity pass ----
            rec = small_pool.tile([C, 1], f32)
            nc.vector.reciprocal(rec[:], allred[:, D_FEAT:D_FEAT + 1])
            cent_bf = persist.tile([C, D_FEAT], bf16)
            nc.vector.tensor_scalar(cent_bf[:], allred[:, 0:D_FEAT], rec[:],
                                    None, MUL)
            centT = persist.tile([128, 2, C], bf16)
            for c in range(2):
                ctp = pstr_pool.tile([128, C], bf16, name="ctp", tag="tp")
                nc.tensor.transpose(ctp[:], cent_bf[:, c * 128:(c + 1) * 128],
                                    ident_bf[:C, :C])
                nc.vector.tensor_copy(centT[:, c, :], ctp[:])

            # ---- pass 2: z = feat @ centT, entropy, weighted accumulation ----
            acc = persist.tile([128, 1], f32)
            nc.vector.memset(acc[:], 0.0)

            for feat, wT in ((sfeat, wT_s), (tfeat, wT_t)):
                fv = feat[:].rearrange("(c b p) d -> c p b d", p=128,
                                       b=CHUNK_BLOCKS)
                for ch in range(NCH):
                    ft = feat_pool.tile([128, CHUNK_BLOCKS, D_FEAT], bf16,
                                        name="ft2", tag="ft1")
                    nc.gpsimd.dma_start(ft[:], fv[ch])
                    zps = psz_pool.tile([128, CHUNK_BLOCKS, 20], f32,
                                        name="zps", tag="zps")
                    for q in range(CHUNK_BLOCKS // 4):
                        tp = pstr_pool.tile([128, 1024], bf16, name="tp2",
                                            tag="tp")
                        for j4 in range(4):
                            j = q * 4 + j4
                            for c in range(2):
                                s = (j4 * 2 + c) * 128
                                nc.tensor.transpose(
                                    tp[:, s:s + 128],
                                    ft[:, j, c * 128:(c + 1) * 128],
                                    ident_bf[:])
                        ftT = ftT_pool.tile([128, 1024], bf16, name="ftT",
                                            tag="ftT")
                        nc.vector.tensor_copy(ftT[:], tp[:])
                        for j4 in range(4):
                            j = q * 4 + j4
                            for c in range(2):
                                s = (j4 * 2 + c) * 128
                                nc.tensor.matmul(zps[:, j, 0:C],
                                                 ftT[:, s:s + 128],
                                                 centT[:, c, :],
                                                 start=(c == 0), stop=(c == 1))
                    # entropy over the supertile
                    zv = zps[:, :, 0:C]  # [128, CB, 19] strided view of PSUM
                    e = ent_pool.tile([128, CHUNK_BLOCKS * C], f32, name="e",
                                      tag="e")
                    e3 = e[:].rearrange("p (a b) -> p a b", b=C)
                    nc.scalar.activation(e[:], zv, Exp)
                    ezz = ent_pool.tile([128, CHUNK_BLOCKS * C], f32,
                                        name="ezz", tag="ezz")
                    ezz3 = ezz[:].rearrange("p (a b) -> p a b", b=C)
                    nc.vector.tensor_tensor(ezz[:], e[:], zv, MUL)
                    S = small_pool.tile([128, CHUNK_BLOCKS], f32, name="S",
                                        tag="S")
                    nc.vector.reduce_sum(S[:], e3, axis=mybir.AxisListType.X)
                    Dd = small_pool.tile([128, CHUNK_BLOCKS], f32, name="Dd",
                                         tag="Dd")
                    nc.vector.reduce_sum(Dd[:], ezz3, axis=mybir.AxisListType.X)
                    logS = small_pool.tile([128, CHUNK_BLOCKS], f32,
                                           name="logS", tag="logS")
                    nc.scalar.activation(logS[:], S[:], Ln)
                    rS = small_pool.tile([128, CHUNK_BLOCKS], f32, name="rS",
                                         tag="rS")
                    nc.vector.reciprocal(rS[:], S[:])
                    ent = small_pool.tile([128, CHUNK_BLOCKS], f32, name="ent",
                                          tag="ent")
                    nc.vector.tensor_tensor(ent[:], Dd[:], rS[:], MUL)
                    nc.vector.tensor_tensor(ent[:], ent[:], logS[:], SUB)
                    nc.vector.tensor_tensor(
                        ent[:], ent[:],
                        wT[:, ch * CHUNK_BLOCKS:(ch + 1) * CHUNK_BLOCKS], MUL)
                    red = small_pool.tile([128, 1], f32, name="red", tag="red")
                    nc.vector.reduce_sum(red[:], ent[:],
                                         axis=mybir.AxisListType.X)
                    nc.vector.tensor_tensor(acc[:], acc[:], red[:], ADD)

            nc.sync.dma_start(accw_out[:], acc[:])

    nc.compile()
    return nc


def get_nc(npix=PIX_PER_CORE, n_cores=N_CORES):
    key = (npix, n_cores)
    if key not in _BUILD_CACHE:
        _BUILD_CACHE[key] = _build(npix, n_cores)
    return _BUILD_CACHE[key]


def make_in_maps(source_feat, target_feat, target_conf, source_argmax,
                 target_argmax, source_mask, n_cores=N_CORES):
    npix = source_feat.shape[0] // n_cores
    maps = []
    for k in range(n_cores):
        s = slice(k * npix, (k + 1) * npix)
        maps.append({
            "sfeat": np.ascontiguousarray(source_feat[s]),
            "tfeat": np.ascontiguousarray(target_feat[s]),
            "tconf": np.ascontiguousarray(target_conf[s]),
            "sam": np.ascontiguousarray(source_argmax[s]),
            "tam": np.ascontiguousarray(target_argmax[s]),
            "smask": np.ascontiguousarray(source_mask[s]).view(np.uint8),
        })
    return maps


def finish_on_host(sred, acc_total, source_mask):
    """sred: [19, 257] allreduced sums; acc_total: sum of all per-core partials."""
    sum_c = sred[:, :D_FEAT]
    denom = sred[:, D_FEAT]
    seen = denom > 0
    cent = np.where(seen[:, None],
                    sum_c / np.maximum(denom, 1e-12)[:, None],
                    np.float32(np.inf)).astype(np.float32)
    n = np.float32(float(source_mask.sum()) + source_mask.shape[0])
    loss = np.float32(-(acc_total / n))
    return np.concatenate([cent.reshape(-1), np.asarray([loss], np.float32)])


def _numpy_reference(source_feat, target_feat, target_conf, source_argmax,
                     target_argmax, source_mask):
    """Exact numpy replica of the reference (fallback for unseen classes)."""
    C = NUM_CLASS
    w_s = source_mask.astype(np.float32)
    w_t = 1.0 - target_conf
    sum_c = np.zeros((C, D_FEAT), np.float32)
    np.add.at(sum_c, source_argmax, source_feat * w_s[:, None])
    np.add.at(sum_c, target_argmax, target_feat * w_t[:, None])
    denom = (np.bincount(source_argmax, weights=w_s, minlength=C)
             + np.bincount(target_argmax, weights=w_t, minlength=C)).astype(
                 np.float32)
    seen = denom > 0
    cent = np.where(seen[:, None], sum_c / np.maximum(denom, 1e-12)[:, None],
                    np.inf).astype(np.float32)
    cent_safe = np.where(seen[:, None], cent, 0.0).astype(np.float32)

    def ent(feat):
        z = feat @ cent_safe.T
        z = np.where(seen[None, :], z, -np.inf)
        zmax = z.max(axis=1, keepdims=True)
        e = np.exp(z - zmax)
        s = e.sum(axis=1, keepdims=True)
        logp = z - (zmax + np.log(s))
        p = e / s
        return np.sum(np.where(seen[None, :], p * logp, 0.0), axis=1)

    total = float((w_s * ent(source_feat)).sum() + (w_t * ent(target_feat)).sum())
    n = float(w_s.sum()) + source_feat.shape[0]
    loss = np.float32(-total / n)
    return np.concatenate([cent.reshape(-1), np.asarray([loss], np.float32)])


def kernel(source_feat, target_feat, target_conf, source_argmax, target_argmax,
           source_mask, _trace=False):
    source_feat = np.asarray(source_feat, np.float32)
    target_feat = np.asarray(target_feat, np.float32)
    target_conf = np.asarray(target_conf, np.float32)
    source_argmax = np.asarray(source_argmax, np.int32)
    target_argmax = np.asarray(target_argmax, np.int32)
    source_mask = np.asarray(source_mask)

    # fast path assumes every class appears with nonzero weight (true for the
    # target distribution with overwhelming probability)
    d_host = (np.bincount(source_argmax, weights=source_mask.astype(np.float64),
                          minlength=NUM_CLASS)
              + np.bincount(target_argmax,
                            weights=(1.0 - target_conf).astype(np.float64),
                            minlength=NUM_CLASS))
    if not np.all(d_host > 0):
        return _numpy_reference(source_feat, target_feat, target_conf,
                                source_argmax, target_argmax, source_mask)

    from concourse.bass_utils import run_bass_kernel_spmd

    nc = get_nc()
    in_maps = make_in_maps(source_feat, target_feat, target_conf, source_argmax,
                           target_argmax, source_mask)
    res = run_bass_kernel_spmd(nc, in_maps, list(range(N_CORES)), trace=_trace)
    sred = res.results[0]["sred"]
    acc_total = float(sum(r["accw"].astype(np.float64).sum()
                          for r in res.results))
    out = finish_on_host(sred, acc_total, source_mask)
    if _trace:
        return out, res
    return out


# revision 9
# speedup vs baseline: 1.0643x; 1.0643x over previous
"""Trainium2 Bass kernel for the ARCS segment-reduce loss (v2).

Math (see reference): per-class weighted segment sums over 2*262144 pixels
-> [19,256] centroids; then z = feat @ cent.T, softmax-entropy per pixel,
confidence-weighted mean -> scalar loss. Output = centroids ++ [loss].

Sharding: data-parallel over pixels, 32768 px/domain/core on 8 cores, with a
[128,39] fp32 AllReduce between the two passes.

Per-core pixel remap: block g, lane p <-> pixel p*B + g (B = 256 blocks).
This is a pure permutation (all reductions are permutation-invariant) chosen
so that every DMA runs long-contiguous and the per-pixel weight / argmax
columns are natural row-major loads (no on-chip transposes for them).

Pass 1 (per 128-px block, all *normal* matmuls so the PE HAM clock-gate
warms up -- transpose-mode ops don't count as PE activity):
  LDW(feat chunk c as stationary [128px, 128d], bf16 cast during DMA)
    MM rhs=onehotw[128,19]   -> accT_c[128d, 19]  (transposed segment sums)
    MM rhs=identity[128,128] -> featT block in PSUM (the pass-2 transpose,
                                 for free off the same weights)
  LDW(onehotw) MM rhs=ones -> acc_d[19,1] (denominators)
onehotw = (iota == argmax_col) * w_col in ONE fused DVE tensor_scalar.
featT blocks: PSUM -> SBUF bf16; target domain stays resident in SBUF
(16 MiB cache), source domain is staged to DRAM and re-read in pass 2.

Pass 2: z[128px,19] = featT.T @ centT accumulated over 2 d-chunks; entropy
uses only free-dim reductions; Ln/reciprocal deferred to one tail pass so
the ACT table is loaded once.
"""

import numpy as np

NUM_CLASS = 19
D_FEAT = 256
N_PIX = 262144
N_CORES = 8
PIX_PER_CORE = N_PIX // N_CORES  # 32768
CB = 16  # blocks per feat DMA chunk / entropy supertile

_BUILD_CACHE = {}


def _build(npix, n_cores):
    import ml_dtypes
    import concourse.bass as bass  # noqa: F401
    import concourse.tile as tile
    from concourse import bacc, mybir

    f32 = mybir.dt.float32
    bf16 = mybir.dt.bfloat16
    EQ = mybir.AluOpType.is_equal
    MUL = mybir.AluOpType.mult
    ADD = mybir.AluOpType.add
    SUB = mybir.AluOpType.subtract
    Exp = mybir.ActivationFunctionType.Exp
    Ln = mybir.ActivationFunctionType.Ln
    X = mybir.AxisListType.X

    C = NUM_CLASS
    B = npix // 128            # blocks per domain (full: 256)
    assert npix % (128 * CB) == 0
    NCH = B // CB              # chunks per domain
    B4 = B // 4                # featT groups of 4 blocks

    nc = bacc.Bacc("TRN2", target_bir_lowering=False, debug=False,
                   num_devices=n_cores)

    sfeat = nc.dram_tensor("sfeat", [npix, D_FEAT], f32, kind="ExternalInput")
    tfeat = nc.dram_tensor("tfeat", [npix, D_FEAT], f32, kind="ExternalInput")
    tconf = nc.dram_tensor("tconf", [npix], f32, kind="ExternalInput")
    sam = nc.dram_tensor("sam", [npix], mybir.dt.int32, kind="ExternalInput")
    tam = nc.dram_tensor("tam", [npix], mybir.dt.int32, kind="ExternalInput")
    smask = nc.dram_tensor("smask", [npix], mybir.dt.uint8, kind="ExternalInput")

    sred_out = nc.dram_tensor("sred", [128, 2 * C + 1], f32,
                              kind="ExternalOutput")
    accw_out = nc.dram_tensor("accw", [128, 1], f32, kind="ExternalOutput")

    ident_bf_d = nc.inline_tensor(np.eye(128).astype(ml_dtypes.bfloat16),
                                  "ident_bf")
    ident_f32_d = nc.inline_tensor(np.eye(128, dtype=np.float32), "ident_f32")
    iota_d = nc.inline_tensor(np.tile(np.arange(C, dtype=np.float32), (128, 1)),
                              "iota_c")

    with tile.TileContext(nc) as tc:
        with (
            tc.tile_pool(name="const", bufs=1) as const_pool,
            tc.tile_pool(name="persist", bufs=1) as persist,
            tc.tile_pool(name="cache", bufs=1) as cache_pool,
            tc.tile_pool(name="feat", bufs=3) as feat_pool,
            tc.tile_pool(name="oh", bufs=8) as oh_pool,
            tc.tile_pool(name="sc", bufs=4) as sc_pool,
            tc.tile_pool(name="ld", bufs=3) as ld_pool,
            tc.tile_pool(name="ent", bufs=2) as ent_pool,
            tc.tile_pool(name="small", bufs=1) as small_pool,
            tc.tile_pool(name="psacc", bufs=1, space="PSUM") as psacc_pool,
            tc.tile_pool(name="pstr", bufs=3, space="PSUM") as pstr_pool,
            tc.tile_pool(name="psz", bufs=2, space="PSUM") as psz_pool,
            tc.tile_pool(name="dram", bufs=1, space="DRAM") as dram_pool,
        ):
            ident_bf = const_pool.tile([128, 128], bf16)
            nc.sync.dma_start(ident_bf[:], ident_bf_d[:])
            ident_f32 = const_pool.tile([128, 128], f32)
            nc.sync.dma_start(ident_f32[:], ident_f32_d[:])
            iota = const_pool.tile([128, C], f32)
            nc.sync.dma_start(iota[:], iota_d[:])
            ones_bf = const_pool.tile([128, 1], bf16)
            nc.vector.memset(ones_bf[:], 1.0)
            ones_f32r = const_pool.tile([1, 128], f32)
            nc.vector.memset(ones_f32r[:], 1.0)

            # per-pixel weights / argmax: natural row-major loads, wT[p, g]
            # = w[p*B + g]
            wT_s = persist.tile([128, B], f32)
            raw_m = small_pool.tile([128, B], mybir.dt.uint8, name="raw_m")
            nc.sync.dma_start(raw_m[:], smask[:].rearrange("(p g) -> p g", g=B))
            nc.vector.tensor_copy(wT_s[:], raw_m[:])
            wT_t = persist.tile([128, B], f32)
            nc.sync.dma_start(wT_t[:], tconf[:].rearrange("(p g) -> p g", g=B))
            nc.vector.tensor_scalar(wT_t[:], wT_t[:], -1.0, 1.0, MUL, ADD)
            amT_s = persist.tile([128, B], f32)
            raw_s = small_pool.tile([128, B], mybir.dt.int32, name="raw_s")
            nc.sync.dma_start(raw_s[:], sam[:].rearrange("(p g) -> p g", g=B))
            nc.vector.tensor_copy(amT_s[:], raw_s[:])
            amT_t = persist.tile([128, B], f32)
            raw_t = small_pool.tile([128, B], mybir.dt.int32, name="raw_t")
            nc.sync.dma_start(raw_t[:], tam[:].rearrange("(p g) -> p g", g=B))
            nc.vector.tensor_copy(amT_t[:], raw_t[:])

            # persistent accumulators
            accT0 = psacc_pool.tile([128, C], f32)
            accT1 = psacc_pool.tile([128, C], f32)
            acc_d = psacc_pool.tile([C, 1], f32)
            tgt_cache = cache_pool.tile([128, B4, 1024], bf16)
            stg = dram_pool.tile([B4, 128, 1024], bf16)
            S_all = persist.tile([128, 2 * B], f32)
            D_all = persist.tile([128, 2 * B], f32)

            # ---------------- pass 1 ----------------
            first = True
            for dom, (feat, amT, wT) in enumerate(
                    ((sfeat, amT_s, wT_s), (tfeat, amT_t, wT_t))):
                fv = feat[:].rearrange("(p c b) d -> c p b d", c=NCH, b=CB)
                for ch in range(NCH):
                    ft = feat_pool.tile([128, CB, D_FEAT], bf16, name="ft1",
                                        tag="ft1")
                    nc.gpsimd.dma_start(ft[:], fv[ch])
                    for jq in range(CB // 4):  # featT groups of 4 blocks
                        bankA = pstr_pool.tile([128, 4, 128], f32, name="bankA",
                                               tag="bank")
                        bankB = pstr_pool.tile([128, 4, 128], f32, name="bankB",
                                               tag="bank")
                        for j4 in range(4):
                            j = jq * 4 + j4
                            g = ch * CB + j
                            last = (dom == 1 and ch == NCH - 1 and j == CB - 1)
                            oh = oh_pool.tile([128, C], bf16, name="oh",
                                              tag="oh")
                            nc.vector.tensor_scalar(oh[:], iota[:],
                                                    amT[:, g:g + 1],
                                                    wT[:, g:g + 1], EQ, MUL)
                            bank = bankA if j4 < 2 else bankB
                            for c in range(2):
                                fslice = ft[:, j, c * 128:(c + 1) * 128]
                                accT = accT0 if c == 0 else accT1
                                nc.tensor.matmul(accT[:], fslice, oh[:],
                                                 start=first, stop=last)
                                nc.tensor.matmul(bank[:, (j4 % 2) * 2 + c, :],
                                                 fslice, ident_bf[:],
                                                 start=True, stop=True)
                            nc.tensor.matmul(acc_d[:], oh[:], ones_bf[:],
                                             start=first, stop=last)
                            first = False
                        # evacuate featT: 4 blocks -> one [128,1024] bf16 tile
                        g0 = ch * CB + jq * 4
                        if dom == 1:
                            dst = tgt_cache[:, g0 // 4, :]
                            nc.vector.tensor_copy(dst[0:128, 0:512], bankA[:])
                            nc.scalar.copy(dst[0:128, 512:1024], bankB[:])
                        else:
                            sc = sc_pool.tile([128, 1024], bf16, name="sc",
                                              tag="sc")
                            nc.vector.tensor_copy(sc[:, 0:512], bankA[:])
                            nc.scalar.copy(sc[:, 512:1024], bankB[:])
                            nc.sync.dma_start(stg[g0 // 4], sc[:])

            # ---------------- AllReduce [128, 39] ----------------
            cc_sb = persist.tile([128, 2 * C + 1], f32)
            nc.vector.tensor_copy(cc_sb[:, 0:C], accT0[:])
            nc.vector.tensor_copy(cc_sb[:, C:2 * C], accT1[:])
            nc.vector.memset(cc_sb[:, 2 * C:2 * C + 1], 0.0)
            nc.vector.tensor_copy(cc_sb[0:C, 2 * C:2 * C + 1], acc_d[:])
            cc_in = dram_pool.tile([128, 2 * C + 1], f32)
            cc_addr = "Shared" if n_cores > 4 else "Local"
            cc_out = dram_pool.tile([128, 2 * C + 1], f32, addr_space=cc_addr)
            nc.gpsimd.dma_start(cc_in[:], cc_sb[:])
            nc.gpsimd.collective_compute(
                "AllReduce", mybir.AluOpType.add,
                replica_groups=[list(range(n_cores))],
                ins=[cc_in.opt()], outs=[cc_out.opt()])
            allred = persist.tile([128, 2 * C + 1], f32)
            nc.gpsimd.dma_start(allred[:], cc_out[:])
            nc.sync.dma_start(sred_out[:], allred[:])

            # centT[d, c] = accT[d, c] / denom[c]  (bf16, for the z matmuls)
            rec_col = small_pool.tile([C, 1], f32, name="rec_col")
            nc.vector.reciprocal(rec_col[:], allred[0:C, 2 * C:2 * C + 1])
            rec_ps = pstr_pool.tile([1, C], f32, name="rec_ps", tag="bank")
            nc.tensor.transpose(rec_ps[:], rec_col[:], ident_f32[0:C, 0:C])
            rec_row = small_pool.tile([1, C], f32, name="rec_row")
            nc.vector.tensor_copy(rec_row[:], rec_ps[:])
            recb_ps = pstr_pool.tile([128, C], f32, name="recb_ps", tag="bank")
            nc.tensor.matmul(recb_ps[:], ones_f32r[:], rec_row[:],
                             start=True, stop=True)
            rec_tile = small_pool.tile([128, C], f32, name="rec_tile")
            nc.vector.tensor_copy(rec_tile[:], recb_ps[:])
            centT = persist.tile([128, 2, C], bf16)
            nc.vector.tensor_tensor(centT[:, 0, :], allred[:, 0:C], rec_tile[:],
                                    MUL)
            nc.vector.tensor_tensor(centT[:, 1, :], allred[:, C:2 * C],
                                    rec_tile[:], MUL)

            # ---------------- pass 2 ----------------
            for dom in (1, 0):  # target (cached) first, then source (staged)
                for ch in range(NCH):
                    if dom == 0:
                        # one 1 MiB DMA per 4 staged tiles (16 blocks/chunk)
                        ldt = ld_pool.tile([128, 4, 1024], bf16, name="ldt",
                                           tag="ldt")
                        t0 = (ch * CB) // 4
                        nc.sync.dma_start(
                            ldt[:],
                            stg[t0:t0 + 4].rearrange("t p x -> p t x"))
                    zps = psz_pool.tile([128, CB, 20], f32, name="zps",
                                        tag="zps")
                    for j in range(CB):
                        g = ch * CB + j
                        for c in range(2):
                            if dom == 1:
                                lhsT = tgt_cache[:, g // 4,
                                                 ((g % 4) * 2 + c) * 128:
                                                 ((g % 4) * 2 + c + 1) * 128]
                            else:
                                s = ((g % 4) * 2 + c) * 128
                                lhsT = ldt[:, (g % 16) // 4, s:s + 128]
                            nc.tensor.matmul(zps[:, j, 0:C], lhsT,
                                             centT[:, c, :],
                                             start=(c == 0), stop=(c == 1))
                    zv = zps[:, :, 0:C]
                    e = ent_pool.tile([128, CB * C], f32, name="e", tag="e")
                    nc.scalar.activation(e[:], zv, Exp)
                    ezz = ent_pool.tile([128, CB * C], f32, name="ezz",
                                        tag="ezz")
                    nc.vector.tensor_tensor(ezz[:], e[:], zv, MUL)
                    col = dom * B + ch * CB
                    nc.vector.reduce_sum(S_all[:, col:col + CB],
                                         e[:].rearrange("p (a b) -> p a b",
                                                        b=C), axis=X)
                    nc.vector.reduce_sum(D_all[:, col:col + CB],
                                         ezz[:].rearrange("p (a b) -> p a b",
                                                          b=C), axis=X)

            # ---------------- tail: ent = (D/S - ln S) * w ----------------
            logS = persist.tile([128, 2 * B], f32)
            nc.scalar.activation(logS[:], S_all[:], Ln)
            rS = persist.tile([128, 2 * B], f32)
            nc.vector.reciprocal(rS[:], S_all[:])
            ent_all = persist.tile([128, 2 * B], f32)
            nc.vector.tensor_tensor(ent_all[:], D_all[:], rS[:], MUL)
            nc.vector.tensor_tensor(ent_all[:], ent_all[:], logS[:], SUB)
            nc.vector.tensor_tensor(ent_all[:, 0:B], ent_all[:, 0:B], wT_s[:],
                                    MUL)
            nc.vector.tensor_tensor(ent_all[:, B:2 * B], ent_all[:, B:2 * B],
                                    wT_t[:], MUL)
            acc = persist.tile([128, 1], f32)
            nc.vector.reduce_sum(acc[:], ent_all[:], axis=X)
            nc.sync.dma_start(accw_out[:], acc[:])

    nc.compile()
    return nc


def get_nc(npix=PIX_PER_CORE, n_cores=N_CORES):
    key = (npix, n_cores)
    if key not in _BUILD_CACHE:
        _BUILD_CACHE[key] = _build(npix, n_cores)
    return _BUILD_CACHE[key]


def make_in_maps(source_feat, target_feat, target_conf, source_argmax,
                 target_argmax, source_mask, n_cores=N_CORES):
    npix = source_feat.shape[0] // n_cores
    maps = []
    for k in range(n_cores):
        s = slice(k * npix, (k + 1) * npix)
        maps.append({
            "sfeat": np.ascontiguousarray(source_feat[s]),
            "tfeat": np.ascontiguousarray(target_feat[s]),
            "tconf": np.ascontiguousarray(target_conf[s]),
            "sam": np.ascontiguousarray(source_argmax[s]),
            "tam": np.ascontiguousarray(target_argmax[s]),
            "smask": np.ascontiguousarray(source_mask[s]).view(np.uint8),
        })
    return maps


def finish_on_host(sred, acc_total, source_mask):
    """sred: [128, 39] allreduced (accT0 | accT1 | denom col)."""
    C = NUM_CLASS
    sum_c = np.concatenate([sred[:, 0:C], sred[:, C:2 * C]], axis=0).T
    denom = sred[0:C, 2 * C]
    seen = denom > 0
    cent = np.where(seen[:, None],
                    sum_c / np.maximum(denom, 1e-12)[:, None],
                    np.float32(np.inf)).astype(np.float32)
    n = np.float32(float(source_mask.sum()) + source_mask.shape[0])
    loss = np.float32(-(acc_total / n))
    return np.concatenate([cent.reshape(-1), np.asarray([loss], np.float32)])


def _numpy_reference(source_feat, target_feat, target_conf, source_argmax,
                     target_argmax, source_mask):
    """Exact numpy replica of the reference (fallback path)."""
    C = NUM_CLASS
    w_s = source_mask.astype(np.float32)
    w_t = 1.0 - target_conf
    sum_c = np.zeros((C, D_FEAT), np.float32)
    np.add.at(sum_c, source_argmax, source_feat * w_s[:, None])
    np.add.at(sum_c, target_argmax, target_feat * w_t[:, None])
    denom = (np.bincount(source_argmax, weights=w_s, minlength=C)
             + np.bincount(target_argmax, weights=w_t, minlength=C)).astype(
                 np.float32)
    seen = denom > 0
    cent = np.where(seen[:, None], sum_c / np.maximum(denom, 1e-12)[:, None],
                    np.inf).astype(np.float32)
    cent_safe = np.where(seen[:, None], cent, 0.0).astype(np.float32)

    def ent(feat):
        z = feat @ cent_safe.T
        z = np.where(seen[None, :], z, -np.inf)
        zmax = z.max(axis=1, keepdims=True)
        e = np.exp(z - zmax)
        s = e.sum(axis=1, keepdims=True)
        logp = z - (zmax + np.log(s))
        p = e / s
        return np.sum(np.where(seen[None, :], p * logp, 0.0), axis=1)

    total = float((w_s * ent(source_feat)).sum()
                  + (w_t * ent(target_feat)).sum())
    n = float(w_s.sum()) + source_feat.shape[0]
    loss = np.float32(-total / n)
    return np.concatenate([cent.reshape(-1), np.asarray([loss], np.float32)])


def kernel(source_feat, target_feat, target_conf, source_argmax, target_argmax,
           source_mask, _trace=False):
    source_feat = np.asarray(source_feat, np.float32)
    target_feat = np.asarray(target_feat, np.float32)
    target_conf = np.asarray(target_conf, np.float32)
    source_argmax = np.asarray(source_argmax, np.int32)
    target_argmax = np.asarray(target_argmax, np.int32)
    source_mask = np.asarray(source_mask)

    d_host = (np.bincount(source_argmax,
                          weights=source_mask.astype(np.float64),
                          minlength=NUM_CLASS)
              + np.bincount(target_argmax,
                            weights=(1.0 - target_conf).astype(np.float64),
                            minlength=NUM_CLASS))
    if not np.all(d_host > 0):
        return _numpy_reference(source_feat, target_feat, target_conf,
                                source_argmax, target_argmax, source_mask)

    from concourse.bass_utils import run_bass_kernel_spmd

    nc = get_nc()
    in_maps = make_in_maps(source_feat, target_feat, target_conf,
                           source_argmax, target_argmax, source_mask)
    res = run_bass_kernel_spmd(nc, in_maps, list(range(N_CORES)),
                               trace=_trace)
    sred = res.results[0]["sred"]
    acc_total = float(sum(r["accw"].astype(np.float64).sum()
                          for r in res.results))
    out = finish_on_host(sred, acc_total, source_mask)
    if _trace:
        return out, res
    return out


# revision 12
# speedup vs baseline: 1.2549x; 1.1791x over previous
"""Trainium2 Bass kernel for the ARCS segment-reduce loss (v2).

Math (see reference): per-class weighted segment sums over 2*262144 pixels
-> [19,256] centroids; then z = feat @ cent.T, softmax-entropy per pixel,
confidence-weighted mean -> scalar loss. Output = centroids ++ [loss].

Sharding: data-parallel over pixels, 32768 px/domain/core on 8 cores, with a
[128,39] fp32 AllReduce between the two passes.

Per-core pixel remap: block g, lane p <-> pixel p*B + g (B = 256 blocks).
This is a pure permutation (all reductions are permutation-invariant) chosen
so that every DMA runs long-contiguous and the per-pixel weight / argmax
columns are natural row-major loads (no on-chip transposes for them).

Pass 1 (per 128-px block, all *normal* matmuls so the PE HAM clock-gate
warms up -- transpose-mode ops don't count as PE activity):
  LDW(feat chunk c as stationary [128px, 128d], bf16 cast during DMA)
    MM rhs=onehotw[128,19]   -> accT_c[128d, 19]  (transposed segment sums)
    MM rhs=identity[128,128] -> featT block in PSUM (the pass-2 transpose,
                                 for free off the same weights)
  LDW(onehotw) MM rhs=ones -> acc_d[19,1] (denominators)
onehotw = (iota == argmax_col) * w_col in ONE fused DVE tensor_scalar.
featT blocks: PSUM -> SBUF bf16; target domain stays resident in SBUF
(16 MiB cache), source domain is staged to DRAM and re-read in pass 2.

Pass 2: z[128px,19] = featT.T @ centT accumulated over 2 d-chunks; entropy
uses only free-dim reductions; Ln/reciprocal deferred to one tail pass so
the ACT table is loaded once.
"""

import numpy as np

NUM_CLASS = 19
D_FEAT = 256
N_PIX = 262144
N_CORES = 8
PIX_PER_CORE = N_PIX // N_CORES  # 32768
CB = 16  # blocks per feat DMA chunk / entropy supertile

_BUILD_CACHE = {}


def _build(npix, n_cores):
    import ml_dtypes
    import concourse.bass as bass  # noqa: F401
    import concourse.tile as tile
    from concourse import bacc, mybir

    f32 = mybir.dt.float32
    bf16 = mybir.dt.bfloat16
    EQ = mybir.AluOpType.is_equal
    MUL = mybir.AluOpType.mult
    ADD = mybir.AluOpType.add
    SUB = mybir.AluOpType.subtract
    Exp = mybir.ActivationFunctionType.Exp
    Ln = mybir.ActivationFunctionType.Ln
    X = mybir.AxisListType.X

    C = NUM_CLASS
    B = npix // 128            # blocks per domain (full: 256)
    assert npix % (128 * CB) == 0
    NCH = B // CB              # chunks per domain
    B4 = B // 4                # featT groups of 4 blocks

    nc = bacc.Bacc("TRN2", target_bir_lowering=False, debug=False,
                   num_devices=n_cores)

    sfeat = nc.dram_tensor("sfeat", [npix, D_FEAT], f32, kind="ExternalInput")
    tfeat = nc.dram_tensor("tfeat", [npix, D_FEAT], f32, kind="ExternalInput")
    tconf = nc.dram_tensor("tconf", [npix], f32, kind="ExternalInput")
    sam = nc.dram_tensor("sam", [npix], mybir.dt.int32, kind="ExternalInput")
    tam = nc.dram_tensor("tam", [npix], mybir.dt.int32, kind="ExternalInput")
    smask = nc.dram_tensor("smask", [npix], mybir.dt.uint8, kind="ExternalInput")

    sred_out = nc.dram_tensor("sred", [128, 2 * C + 1], f32,
                              kind="ExternalOutput")
    accw_out = nc.dram_tensor("accw", [128, 1], f32, kind="ExternalOutput")

    ident_bf_d = nc.inline_tensor(np.eye(128).astype(ml_dtypes.bfloat16),
                                  "ident_bf")
    ident_f32_d = nc.inline_tensor(np.eye(128, dtype=np.float32), "ident_f32")
    iota_d = nc.inline_tensor(
        np.tile(np.arange(C).astype(ml_dtypes.bfloat16), (128, 1)), "iota_c")

    with tile.TileContext(nc) as tc:
        with (
            tc.tile_pool(name="const", bufs=1) as const_pool,
            tc.tile_pool(name="persist", bufs=1) as persist,
            tc.tile_pool(name="cache", bufs=1) as cache_pool,
            tc.tile_pool(name="feat", bufs=5) as feat_pool,
            tc.tile_pool(name="oh", bufs=8) as oh_pool,
            tc.tile_pool(name="sc", bufs=4) as sc_pool,
            tc.tile_pool(name="ent", bufs=2) as ent_pool,
            tc.tile_pool(name="small", bufs=1) as small_pool,
            tc.tile_pool(name="psacc", bufs=1, space="PSUM") as psacc_pool,
            tc.tile_pool(name="pstr", bufs=3, space="PSUM") as pstr_pool,
            tc.tile_pool(name="psz", bufs=2, space="PSUM") as psz_pool,
            tc.tile_pool(name="dram", bufs=1, space="DRAM") as dram_pool,
        ):
            ident_bf = const_pool.tile([128, 128], bf16)
            nc.sync.dma_start(ident_bf[:], ident_bf_d[:])
            ident_f32 = const_pool.tile([128, 128], f32)
            nc.sync.dma_start(ident_f32[:], ident_f32_d[:])
            iota = const_pool.tile([128, C], bf16)
            nc.sync.dma_start(iota[:], iota_d[:])
            ones_bf = const_pool.tile([128, 1], bf16)
            nc.vector.memset(ones_bf[:], 1.0)
            ones_f32r = const_pool.tile([1, 128], f32)
            nc.vector.memset(ones_f32r[:], 1.0)

            # per-pixel weights / argmax: natural row-major loads, wT[p, g]
            # = w[p*B + g]
            wT_s = persist.tile([128, B], f32)
            raw_m = small_pool.tile([128, B], mybir.dt.uint8, name="raw_m")
            nc.sync.dma_start(raw_m[:], smask[:].rearrange("(p g) -> p g", g=B))
            nc.vector.tensor_copy(wT_s[:], raw_m[:])
            wT_t = persist.tile([128, B], f32)
            raw_c = small_pool.tile([128, B], f32, name="raw_c")
            nc.sync.dma_start(raw_c[:], tconf[:].rearrange("(p g) -> p g", g=B))
            nc.vector.tensor_scalar(wT_t[:], raw_c[:], -1.0, 1.0, MUL, ADD)
            amT_s = persist.tile([128, B], f32)
            raw_s = small_pool.tile([128, B], mybir.dt.int32, name="raw_s")
            nc.sync.dma_start(raw_s[:], sam[:].rearrange("(p g) -> p g", g=B))
            nc.vector.tensor_copy(amT_s[:], raw_s[:])
            amT_t = persist.tile([128, B], f32)
            raw_t = small_pool.tile([128, B], mybir.dt.int32, name="raw_t")
            nc.sync.dma_start(raw_t[:], tam[:].rearrange("(p g) -> p g", g=B))
            nc.vector.tensor_copy(amT_t[:], raw_t[:])

            # persistent accumulators
            accT0 = psacc_pool.tile([128, C], f32)
            accT1 = psacc_pool.tile([128, C], f32)
            acc_d = psacc_pool.tile([C, 1], f32)
            tgt_cache = cache_pool.tile([128, B4, 1024], bf16)
            stg = dram_pool.tile([B4, 128, 1024], bf16)
            S_all = persist.tile([128, 2 * B], f32)
            D_all = persist.tile([128, 2 * B], f32)

            # ---------------- pass 1 ----------------
            warm_ps = pstr_pool.tile([128, 4, 128], f32, name="warm_ps",
                                     tag="bank")
            first = True
            for dom, (feat, amT, wT) in enumerate(
                    ((sfeat, amT_s, wT_s), (tfeat, amT_t, wT_t))):
                fv = feat[:].rearrange("(p c b) d -> c p b d", c=NCH, b=CB)
                for ch in range(NCH):
                    ft = feat_pool.tile([128, CB, D_FEAT], bf16, name="ft1",
                                        tag="ft1")
                    nc.gpsimd.dma_start(ft[:], fv[ch])
                    if dom == 0 and ch == 0:
                        # ~6us dense matmul burst to flip the PE HAM clock
                        # gate to 8/8 before the real (small-N) matmuls
                        for wi in range(20):
                            nc.tensor.matmul(
                                warm_ps[:, 0:2, :], ft[:, wi % CB, 0:128],
                                ft[:, (wi + 1) % CB, :],
                                start=True, stop=True)
                    for jq in range(CB // 4):  # featT groups of 4 blocks
                        bankA = pstr_pool.tile([128, 4, 128], f32, name="bankA",
                                               tag="bank")
                        bankB = pstr_pool.tile([128, 4, 128], f32, name="bankB",
                                               tag="bank")
                        for j4 in range(4):
                            j = jq * 4 + j4
                            g = ch * CB + j
                            last = (dom == 1 and ch == NCH - 1 and j == CB - 1)
                            oh = oh_pool.tile([128, C], bf16, name="oh",
                                              tag="oh")
                            nc.vector.tensor_scalar(oh[:], iota[:],
                                                    amT[:, g:g + 1],
                                                    wT[:, g:g + 1], EQ, MUL)
                            bank = bankA if j4 < 2 else bankB
                            for c in range(2):
                                fslice = ft[:, j, c * 128:(c + 1) * 128]
                                accT = accT0 if c == 0 else accT1
                                nc.tensor.matmul(accT[:], fslice, oh[:],
                                                 start=first, stop=last)
                                nc.tensor.matmul(bank[:, (j4 % 2) * 2 + c, :],
                                                 fslice, ident_bf[:],
                                                 start=True, stop=True)
                            nc.tensor.matmul(acc_d[:], oh[:], ones_bf[:],
                                             start=first, stop=last)
                            first = False
                        # evacuate featT: 4 blocks -> one [128,1024] bf16 tile
                        g0 = ch * CB + jq * 4
                        if dom == 1:
                            dst = tgt_cache[:, g0 // 4, :]
                            nc.vector.tensor_copy(dst[0:128, 0:512], bankA[:])
                            nc.scalar.copy(dst[0:128, 512:1024], bankB[:])
                        else:
                            sc = sc_pool.tile([128, 1024], bf16, name="sc",
                                              tag="sc")
                            nc.vector.tensor_copy(sc[:, 0:512], bankA[:])
                            nc.scalar.copy(sc[:, 512:1024], bankB[:])
                            nc.sync.dma_start(stg[g0 // 4], sc[:])

            # ---------------- AllReduce [128, 39] ----------------
            cc_sb = persist.tile([128, 2 * C + 1], f32)
            nc.vector.tensor_copy(cc_sb[:, 0:C], accT0[:])
            nc.vector.tensor_copy(cc_sb[:, C:2 * C], accT1[:])
            nc.vector.memset(cc_sb[:, 2 * C:2 * C + 1], 0.0)
            nc.vector.tensor_copy(cc_sb[0:C, 2 * C:2 * C + 1], acc_d[:])
            cc_in = dram_pool.tile([128, 2 * C + 1], f32)
            cc_addr = "Shared" if n_cores > 4 else "Local"
            cc_out = dram_pool.tile([128, 2 * C + 1], f32, addr_space=cc_addr)
            nc.gpsimd.dma_start(cc_in[:], cc_sb[:])
            nc.gpsimd.collective_compute(
                "AllReduce", mybir.AluOpType.add,
                replica_groups=[list(range(n_cores))],
                ins=[cc_in.opt()], outs=[cc_out.opt()])
            allred = persist.tile([128, 2 * C + 1], f32)
            nc.gpsimd.dma_start(allred[:], cc_out[:])
            nc.sync.dma_start(sred_out[:], allred[:])

            # centT[d, c] = accT[d, c] / denom[c]  (bf16, for the z matmuls)
            rec_col = small_pool.tile([C, 1], f32, name="rec_col")
            nc.vector.reciprocal(rec_col[:], allred[0:C, 2 * C:2 * C + 1])
            rec_ps = pstr_pool.tile([1, C], f32, name="rec_ps", tag="bank")
            nc.tensor.transpose(rec_ps[:], rec_col[:], ident_f32[0:C, 0:C])
            rec_row = small_pool.tile([1, C], f32, name="rec_row")
            nc.vector.tensor_copy(rec_row[:], rec_ps[:])
            recb_ps = pstr_pool.tile([128, C], f32, name="recb_ps", tag="bank")
            nc.tensor.matmul(recb_ps[:], ones_f32r[:], rec_row[:],
                             start=True, stop=True)
            rec_tile = small_pool.tile([128, C], f32, name="rec_tile")
            nc.vector.tensor_copy(rec_tile[:], recb_ps[:])
            centT = persist.tile([128, 2, C], bf16)
            nc.vector.tensor_tensor(centT[:, 0, :], allred[:, 0:C], rec_tile[:],
                                    MUL)
            nc.vector.tensor_tensor(centT[:, 1, :], allred[:, C:2 * C],
                                    rec_tile[:], MUL)

            # ---------------- pass 2 ----------------
            for dom in (0, 1):  # source first: staged loads prefetch during AR
                for ch in range(NCH):
                    if dom == 0:
                        # one 1 MiB DMA per 4 staged tiles (16 blocks/chunk)
                        ldt = feat_pool.tile([128, 4, 1024], bf16, name="ldt",
                                           tag="ft1")
                        t0 = (ch * CB) // 4
                        nc.sync.dma_start(
                            ldt[:],
                            stg[t0:t0 + 4].rearrange("t p x -> p t x"))
                    zps = psz_pool.tile([128, CB, 20], f32, name="zps",
                                        tag="zps")
                    for j in range(CB):
                        g = ch * CB + j
                        for c in range(2):
                            if dom == 1:
                                lhsT = tgt_cache[:, g // 4,
                                                 ((g % 4) * 2 + c) * 128:
                                                 ((g % 4) * 2 + c + 1) * 128]
                            else:
                                s = ((g % 4) * 2 + c) * 128
                                lhsT = ldt[:, (g % 16) // 4, s:s + 128]
                            nc.tensor.matmul(zps[:, j, 0:C], lhsT,
                                             centT[:, c, :],
                                             start=(c == 0), stop=(c == 1))
                    zv = zps[:, :, 0:C]
                    e = ent_pool.tile([128, CB * C], f32, name="e", tag="e")
                    nc.scalar.activation(e[:], zv, Exp)
                    ezz = ent_pool.tile([128, CB * C], f32, name="ezz",
                                        tag="ezz")
                    nc.vector.tensor_tensor(ezz[:], e[:], zv, MUL)
                    col = dom * B + ch * CB
                    nc.vector.reduce_sum(S_all[:, col:col + CB],
                                         e[:].rearrange("p (a b) -> p a b",
                                                        b=C), axis=X)
                    nc.vector.reduce_sum(D_all[:, col:col + CB],
                                         ezz[:].rearrange("p (a b) -> p a b",
                                                          b=C), axis=X)

            # ---------------- tail: ent = (D/S - ln S) * w ----------------
            logS = persist.tile([128, 2 * B], f32)
            nc.scalar.activation(logS[:], S_all[:], Ln)
            rS = persist.tile([128, 2 * B], f32)
            nc.vector.reciprocal(rS[:], S_all[:])
            ent_all = persist.tile([128, 2 * B], f32)
            nc.vector.tensor_tensor(ent_all[:], D_all[:], rS[:], MUL)
            nc.vector.tensor_tensor(ent_all[:], ent_all[:], logS[:], SUB)
            nc.vector.tensor_tensor(ent_all[:, 0:B], ent_all[:, 0:B], wT_s[:],
                                    MUL)
            nc.vector.tensor_tensor(ent_all[:, B:2 * B], ent_all[:, B:2 * B],
                                    wT_t[:], MUL)
            acc = persist.tile([128, 1], f32)
            nc.vector.reduce_sum(acc[:], ent_all[:], axis=X)
            nc.sync.dma_start(accw_out[:], acc[:])

    nc.compile()
    return nc


def get_nc(npix=PIX_PER_CORE, n_cores=N_CORES):
    key = (npix, n_cores)
    if key not in _BUILD_CACHE:
        _BUILD_CACHE[key] = _build(npix, n_cores)
    return _BUILD_CACHE[key]


def make_in_maps(source_feat, target_feat, target_conf, source_argmax,
                 target_argmax, source_mask, n_cores=N_CORES):
    npix = source_feat.shape[0] // n_cores
    maps = []
    for k in range(n_cores):
        s = slice(k * npix, (k + 1) * npix)
        maps.append({
            "sfeat": np.ascontiguousarray(source_feat[s]),
            "tfeat": np.ascontiguousarray(target_feat[s]),
            "tconf": np.ascontiguousarray(target_conf[s]),
            "sam": np.ascontiguousarray(source_argmax[s]),
            "tam": np.ascontiguousarray(target_argmax[s]),
            "smask": np.ascontiguousarray(source_mask[s]).view(np.uint8),
        })
    return maps


def finish_on_host(sred, acc_total, source_mask):
    """sred: [128, 39] allreduced (accT0 | accT1 | denom col)."""
    C = NUM_CLASS
    sum_c = np.concatenate([sred[:, 0:C], sred[:, C:2 * C]], axis=0).T
    denom = sred[0:C, 2 * C]
    seen = denom > 0
    cent = np.where(seen[:, None],
                    sum_c / np.maximum(denom, 1e-12)[:, None],
                    np.float32(np.inf)).astype(np.float32)
    n = np.float32(float(source_mask.sum()) + source_mask.shape[0])
    loss = np.float32(-(acc_total / n))
    return np.concatenate([cent.reshape(-1), np.asarray([loss], np.float32)])


def _numpy_reference(source_feat, target_feat, target_conf, source_argmax,
                     target_argmax, source_mask):
    """Exact numpy replica of the reference (fallback path)."""
    C = NUM_CLASS
    w_s = source_mask.astype(np.float32)
    w_t = 1.0 - target_conf
    sum_c = np.zeros((C, D_FEAT), np.float32)
    np.add.at(sum_c, source_argmax, source_feat * w_s[:, None])
    np.add.at(sum_c, target_argmax, target_feat * w_t[:, None])
    denom = (np.bincount(source_argmax, weights=w_s, minlength=C)
             + np.bincount(target_argmax, weights=w_t, minlength=C)).astype(
                 np.float32)
    seen = denom > 0
    cent = np.where(seen[:, None], sum_c / np.maximum(denom, 1e-12)[:, None],
                    np.inf).astype(np.float32)
    cent_safe = np.where(seen[:, None], cent, 0.0).astype(np.float32)

    def ent(feat):
        z = feat @ cent_safe.T
        z = np.where(seen[None, :], z, -np.inf)
        zmax = z.max(axis=1, keepdims=True)
        e = np.exp(z - zmax)
        s = e.sum(axis=1, keepdims=True)
        logp = z - (zmax + np.log(s))
        p = e / s
        return np.sum(np.where(seen[None, :], p * logp, 0.0), axis=1)

    total = float((w_s * ent(source_feat)).sum()
                  + (w_t * ent(target_feat)).sum())
    n = float(w_s.sum()) + source_feat.shape[0]
    loss = np.float32(-total / n)
    return np.concatenate([cent.reshape(-1), np.asarray([loss], np.float32)])


def kernel(source_feat, target_feat, target_conf, source_argmax, target_argmax,
           source_mask, _trace=False):
    source_feat = np.asarray(source_feat, np.float32)
    target_feat = np.asarray(target_feat, np.float32)
    target_conf = np.asarray(target_conf, np.float32)
    source_argmax = np.asarray(source_argmax, np.int32)
    target_argmax = np.asarray(target_argmax, np.int32)
    source_mask = np.asarray(source_mask)

    d_host = (np.bincount(source_argmax,
                          weights=source_mask.astype(np.float64),
                          minlength=NUM_CLASS)
              + np.bincount(target_argmax,
                            weights=(1.0 - target_conf).astype(np.float64),
                            minlength=NUM_CLASS))
    if not np.all(d_host > 0):
        return _numpy_reference(source_feat, target_feat, target_conf,
                                source_argmax, target_argmax, source_mask)

    from concourse.bass_utils import run_bass_kernel_spmd

    nc = get_nc()
    in_maps = make_in_maps(source_feat, target_feat, target_conf,
                           source_argmax, target_argmax, source_mask)
    res = run_bass_kernel_spmd(nc, in_maps, list(range(N_CORES)),
                               trace=_trace)
    sred = res.results[0]["sred"]
    acc_total = float(sum(r["accw"].astype(np.float64).sum()
                          for r in res.results))
    out = finish_on_host(sred, acc_total, source_mask)
    if _trace:
        return out, res
    return out
